# revision 1
# baseline (speedup 1.0000x reference)
"""Trainium2 Bass kernel for nn_MultiHeadAttention_55894704390646.

Multi-head causal attention, B=2, S=2048, E=1024, H=16 heads, D=64.
Sharding: data-parallel over batch (2 groups) x tensor-parallel over heads
(4 heads per core). Each core computes a partial output-projection result
(row-split Wo); the host sums the 4 partials per batch and adds the bias.

Device-side dataflow (per core, all fp32 with float32r matmuls):
  - host supplies x^T [E, S] and pre-transposed weight slices, so every
    matmul contraction dim lands on SBUF partitions with no on-device
    transposes.
  - qT/kT computed in [d, tokens] layout, v in [tokens, d] layout.
  - scores computed transposed ([keys, queries]); softmax uses
    exp(s/8) with a multiplicative causal mask (no max subtraction needed:
    |s/8| is bounded by ~±6) and the denominator comes from a ones-column
    appended to v (M=65 matmul).
  - normalization multiplies by 1/denom broadcast across partitions via a
    PE outer product.
"""

import sys

if "/opt/trn_rl_repo" not in sys.path:
    sys.path.insert(0, "/opt/trn_rl_repo")

import numpy as np

import concourse.bass as bass
from concourse import bacc
import concourse.mybir as mybir
import concourse.tile as tile
from concourse.bass_utils import run_bass_kernel_spmd

B, S, E, H, D = 2, 2048, 1024, 16, 64
N_CORES = 8
DP = 2                 # batch groups
TP = 4                 # cores per batch group
HL = H // TP           # local heads per core = 4
DL = HL * D            # local head dims = 256
P = 128
NTB = S // P           # token blocks = 16
QC = 512               # query chunk
NQC = S // QC          # query chunks = 4
NKB_PER_QC = QC // P   # k-blocks per q chunk = 4
NPAIR = HL // 2        # head pairs = 2
NEO = E // QC          # output feature chunks of 512 = 2
NKO = E // P           # contraction blocks over E = 8

f32 = mybir.dt.float32
f32r = mybir.dt.float32r
EXP = mybir.ActivationFunctionType.Exp

_NC_CACHE = None


def _build_nc():
    nc = bacc.Bacc("TRN2", target_bir_lowering=False, debug=False)

    xT = nc.dram_tensor("xT", (E, S), f32r, kind="ExternalInput")
    wqT = nc.dram_tensor("wqT", (E, DL), f32r, kind="ExternalInput")
    wkT = nc.dram_tensor("wkT", (E, DL), f32r, kind="ExternalInput")
    wvT = nc.dram_tensor("wvT", (E, DL), f32r, kind="ExternalInput")
    woT = nc.dram_tensor("woT", (DL, E), f32r, kind="ExternalInput")
    out = nc.dram_tensor("out", (S, E), f32, kind="ExternalOutput")

    with tile.TileContext(nc) as tc:
        with (
            nc.allow_low_precision(reason="float32r is the intended matmul dtype"),
            tc.tile_pool(name="big", bufs=1) as big,
            tc.tile_pool(name="work", bufs=4) as work,
            tc.tile_pool(name="work2", bufs=2) as work2,
            tc.tile_pool(name="ps", bufs=2, space="PSUM") as ps,
            tc.tile_pool(name="ps_s", bufs=2, space="PSUM") as ps_s,
            tc.tile_pool(name="ps_ctx", bufs=2, space="PSUM") as ps_ctx,
        ):
            # ---- loads: q/k weights, then x^T per k-block, then v/o weights.
            # All persistent tensors are chunk-granular tiles so the Tile
            # scheduler sees fine-grained deps and can overlap phases.
            wqT_sb = big.tile([P, NKO, DL], f32r, tag="wqT")
            nc.sync.dma_start(wqT_sb[:], wqT[:].rearrange("(ko p) d -> p ko d", p=P))
            wkT_sb = big.tile([P, NKO, DL], f32r, tag="wkT")
            nc.sync.dma_start(wkT_sb[:], wkT[:].rearrange("(ko p) d -> p ko d", p=P))
            # x^T is loaded as 32 (ko, token-chunk) tiles so early projection
            # groups complete before the whole 8.4MB input lands.
            xT_r = xT[:].rearrange("(ko p) (c s) -> p ko c s", p=P, c=NQC)
            xT_q = [
                [
                    big.tile([P, QC], f32r, tag=f"xTq{ko}_{c}", name=f"xTq{ko}_{c}")
                    for c in range(NQC)
                ]
                for ko in range(NKO)
            ]
            for c in range(NQC):
                for ko in range(NKO):
                    nc.sync.dma_start(xT_q[ko][c][:], xT_r[:, ko, c, :])
                if c == 0:
                    wvT_sb = big.tile([P, NKO, DL], f32r, tag="wvT")
                    nc.sync.dma_start(
                        wvT_sb[:], wvT[:].rearrange("(ko p) d -> p ko d", p=P)
                    )
            woT_sb = big.tile([P, NPAIR, E], f32r, tag="woT")
            nc.sync.dma_start(woT_sb[:], woT[:].rearrange("(pr p) e -> p pr e", p=P))

            # causal masks for the 4 diagonal-region k-blocks of a q-chunk:
            # mask_i[k, q] = 1 if (k + 128*i) <= q else 0
            masks = []
            for i in range(NKB_PER_QC):
                m = big.tile([P, QC], mybir.dt.bfloat16, tag=f"mask{i}", name=f"mask{i}")
                nc.gpsimd.memset(m[:], 1.0)
                nc.gpsimd.affine_select(
                    out=m[:],
                    in_=m[:],
                    compare_op=mybir.AluOpType.is_ge,
                    fill=0.0,
                    base=-P * i,
                    pattern=[[1, QC]],
                    channel_multiplier=-1,
                )
                masks.append(m)

            ones_stage = big.tile([P, HL], f32, tag="ones_stage")
            nc.gpsimd.memset(ones_stage[:], 1.0)

            # chunk-granular persistent activation buffers
            qT_c = [[None] * NQC for _ in range(NPAIR)]
            kT_c = [[None] * NQC for _ in range(NPAIR)]
            for pr in range(NPAIR):
                for ch in range(NQC):
                    qT_c[pr][ch] = big.tile(
                        [P, QC], f32r, tag=f"qT{pr}{ch}", name=f"qT{pr}{ch}"
                    )
                    kT_c[pr][ch] = big.tile(
                        [P, QC], f32r, tag=f"kT{pr}{ch}", name=f"kT{pr}{ch}"
                    )
            v_tb = []
            for tb in range(NTB):
                vt = big.tile([P, HL, D + 1], f32r, tag=f"v{tb}", name=f"v{tb}")
                nc.vector.tensor_copy(vt[:, :, D], ones_stage[:, :])
                v_tb.append(vt)
            ctx_J = []
            for J in range(NQC):
                ctx_J.append(
                    big.tile([P, NPAIR, QC], f32r, tag=f"ctxT{J}", name=f"ctxT{J}")
                )

            def emit_qk_ch(ch):
                """qT/kT projections for one token chunk, all pairs."""
                for pr in range(NPAIR):
                    for wt_sb, dst in ((wqT_sb, qT_c), (wkT_sb, kT_c)):
                        pp = ps.tile([P, QC], f32, tag="mm", name=f"pp_{pr}_{ch}")
                        for ko in range(NKO):
                            nc.tensor.matmul(
                                pp[:],
                                wt_sb[:, ko, pr * P : (pr + 1) * P],
                                xT_q[ko][ch][:],
                                start=(ko == 0),
                                stop=(ko == NKO - 1),
                            )
                        nc.scalar.copy(dst[pr][ch][:], pp[:])

            def emit_v(tb0, tb1):
                for tb in range(tb0, tb1):
                    pv_full = ps.tile([P, QC], f32, tag="mm", name="pv_full")
                    pv = pv_full[:, 0:DL]
                    for ko in range(NKO):
                        nc.tensor.matmul(
                            pv[:],
                            xT_q[ko][tb // NKB_PER_QC][
                                :, (tb % NKB_PER_QC) * P : (tb % NKB_PER_QC + 1) * P
                            ],
                            wvT_sb[:, ko, :],
                            start=(ko == 0),
                            stop=(ko == NKO - 1),
                        )
                    nc.scalar.copy(
                        v_tb[tb][:, :, 0:D],
                        pv[:].rearrange("p (h d) -> p h d", h=HL),
                    )

            def normalize(ctx_ps, pr, r, J):
                """ctxT[h] = ctx[:64] / ctx[64] into its pair slot.

                The PSUM accumulator is evacuated to SBUF immediately (one ACT
                copy) so the bank frees for the next chunk; the reciprocal /
                broadcast / multiply chain then runs off the critical path."""
                cu = work2.tile([D, QC], f32, tag="cu")
                nc.scalar.copy(cu[:], ctx_ps[0:D, :])
                dn = work2.tile([1, QC], f32, tag="nrm", name="dn")
                nc.scalar.copy(dn[:], ctx_ps[D : D + 1, :])
                recip = work2.tile([1, QC], f32, tag="nrm", name="recip")
                nc.vector.reciprocal_approx_fast(recip[:], dn[:])
                dnb = work2.tile([64, QC], f32, tag="dnb")
                nc.gpsimd.partition_broadcast(dnb[:], recip[:])
                if r == 0:
                    nc.vector.tensor_tensor(
                        ctx_J[J][0:64, pr, :],
                        cu[:],
                        dnb[:],
                        mybir.AluOpType.mult,
                    )
                else:
                    tmp = work2.tile([64, QC], f32r, tag="ctmp")
                    nc.vector.tensor_tensor(
                        tmp[:], cu[:], dnb[:], mybir.AluOpType.mult
                    )
                    nc.sync.dma_start(ctx_J[J][64:128, pr, :], tmp[:])

            def attn_scores_group(pr, J, I):
                """Scores + exp + mask for k-block I of pair pr, chunk J."""
                ik = slice((I % NKB_PER_QC) * P, (I % NKB_PER_QC + 1) * P)
                kch = I // NKB_PER_QC
                s = ps_s.tile([P, 2, QC], f32, tag="s", name="s")
                nc.tensor.matmul(
                    s[:, 0, :],
                    kT_c[pr][kch][0:64, ik],
                    qT_c[pr][J][0:64, :],
                    start=True,
                    stop=True,
                )
                nc.tensor.matmul(
                    s[:, 1, :],
                    kT_c[pr][kch][64:128, ik],
                    qT_c[pr][J][64:128, :],
                    start=True,
                    stop=True,
                )
                pT = work.tile([P, 2, QC], f32r, tag="pT", name="pT")
                nc.scalar.activation(pT[:], s[:], EXP, scale=0.125)
                di = I - NKB_PER_QC * J
                if di >= 0:
                    nc.vector.tensor_tensor(
                        pT[:],
                        pT[:],
                        masks[di][:, None, :].to_broadcast((P, 2, QC)),
                        mybir.AluOpType.mult,
                    )
                return pT

            def emit_attn_pair(pr, J):
                """Attention for the head pair (2pr, 2pr+1) on query chunk J.
                The two heads' K=64 score matmuls go back-to-back into the two
                halves of one 2-bank PSUM tile with row groups 0/64, so the PE
                array runs them concurrently. AVs are skewed one k-block behind
                the scores so the in-order PE queue never waits on exp/mask."""
                h0, h1 = 2 * pr, 2 * pr + 1
                nI = NKB_PER_QC * (J + 1)
                ctx0 = ps_ctx.tile([D + 1, QC], f32, tag="ctx", name="ctx0")
                ctx1 = ps_ctx.tile([D + 1, QC], f32, tag="ctx", name="ctx1")

                def emit_av(I, pT):
                    nc.tensor.matmul(
                        ctx0[:], v_tb[I][:, h0, :], pT[:, 0, :],
                        start=(I == 0), stop=(I == nI - 1),
                    )
                    nc.tensor.matmul(
                        ctx1[:], v_tb[I][:, h1, :], pT[:, 1, :],
                        start=(I == 0), stop=(I == nI - 1),
                    )

                prev_pT = pending.pop() if pending else attn_scores_group(pr, J, 0)
                for I in range(1, nI):
                    pT = attn_scores_group(pr, J, I)
                    emit_av(I - 1, prev_pT)
                    prev_pT = pT
                # prefetch the NEXT chunk's first scores group before the last
                # AV + normalize so the PE queue never drains at chunk starts
                nxt = chain.pop(0) if chain else None
                if nxt is not None:
                    pending.append(attn_scores_group(nxt[0], nxt[1], 0))
                emit_av(nI - 1, prev_pT)
                # r=1 head first: its ctx reaches ctx_J via an SBUF shift DMA,
                # so keep that latency off the critical tail
                normalize(ctx1, pr, 1, J)
                normalize(ctx0, pr, 0, J)

            def emit_out(J):
                """Output projection for the token blocks of query chunk J."""
                for tb in range(NKB_PER_QC * J, NKB_PER_QC * (J + 1)):
                    o_sb = work2.tile([P, E], f32, tag="o_sb")
                    tsl = slice((tb % NKB_PER_QC) * P, (tb % NKB_PER_QC + 1) * P)
                    for ec in range(NEO):
                        o_ps = ps.tile([P, QC], f32, tag="mm", name="o_ps")
                        for pr in range(NPAIR):
                            nc.tensor.matmul(
                                o_ps[:],
                                ctx_J[J][:, pr, tsl],
                                woT_sb[:, pr, ec * QC : (ec + 1) * QC],
                                start=(pr == 0),
                                stop=(pr == NPAIR - 1),
                            )
                        nc.vector.tensor_copy(
                            o_sb[:, ec * QC : (ec + 1) * QC], o_ps[:]
                        )
                        nc.sync.dma_start(
                            out[tb * P : (tb + 1) * P, ec * QC : (ec + 1) * QC],
                            o_sb[:, ec * QC : (ec + 1) * QC],
                        )

            chain = [(0, 0), (1, 0), (0, 1), (1, 1), (0, 2), (1, 2), (0, 3), (1, 3)]
            pending = []
            chain.pop(0)
            emit_qk_ch(0)
            emit_v(0, NKB_PER_QC)
            emit_attn_pair(0, 0)
            emit_qk_ch(1)
            emit_attn_pair(1, 0)
            emit_out(0)
            emit_v(NKB_PER_QC, 2 * NKB_PER_QC)
            emit_attn_pair(0, 1)
            emit_qk_ch(2)
            emit_attn_pair(1, 1)
            emit_out(1)
            emit_v(2 * NKB_PER_QC, 3 * NKB_PER_QC)
            emit_attn_pair(0, 2)
            emit_qk_ch(3)
            emit_attn_pair(1, 2)
            emit_out(2)
            emit_v(3 * NKB_PER_QC, NTB)
            emit_attn_pair(0, 3)
            emit_attn_pair(1, 3)
            emit_out(3)

    nc.compile()
    return nc


def get_nc():
    global _NC_CACHE
    if _NC_CACHE is None:
        _NC_CACHE = _build_nc()
    return _NC_CACHE


def _round_fp32r(a):
    """Round-to-nearest-even onto the fp32r grid (11 mantissa bits)."""
    b = np.ascontiguousarray(a, dtype=np.float32).view(np.uint32)
    b = b + 0x7FF + ((b >> 12) & 1)
    b &= np.uint32(0xFFFFF000)
    return b.view(np.float32)


def make_in_maps(x, Wq, Wk, Wv, Wo):
    x = np.asarray(x, dtype=np.float32)
    Wq = np.asarray(Wq, dtype=np.float32)
    Wk = np.asarray(Wk, dtype=np.float32)
    Wv = np.asarray(Wv, dtype=np.float32)
    Wo = np.asarray(Wo, dtype=np.float32)
    in_maps = []
    for c in range(N_CORES):
        b, g = divmod(c, TP)
        sl = slice(DL * g, DL * (g + 1))
        in_maps.append(
            {
                "xT": _round_fp32r(x[b].T),
                "wqT": _round_fp32r(Wq[sl].T),
                "wkT": _round_fp32r(Wk[sl].T),
                "wvT": _round_fp32r(Wv[sl].T),
                "woT": _round_fp32r(Wo[:, sl].T),
            }
        )
    return in_maps


def _combine(results, bo):
    bo = np.asarray(bo, dtype=np.float32)
    y = np.zeros((B, S, E), dtype=np.float32)
    for c in range(N_CORES):
        y[c // TP] += results[c]["out"]
    y += bo
    return y


def kernel(x, Wq, Wk, Wv, Wo, bo):
    nc = get_nc()
    in_maps = make_in_maps(x, Wq, Wk, Wv, Wo)
    res = run_bass_kernel_spmd(nc, in_maps, list(range(N_CORES)))
    return _combine(res.results, bo)


def kernel_traced(x, Wq, Wk, Wv, Wo, bo, trace_cores=None):
    """Like kernel() but with NTFF tracing; returns (output, BassKernelResults)."""
    nc = get_nc()
    in_maps = make_in_maps(x, Wq, Wk, Wv, Wo)
    res = run_bass_kernel_spmd(
        nc, in_maps, list(range(N_CORES)), trace=True, trace_cores=trace_cores
    )
    return _combine(res.results, bo), res



# revision 7
# speedup vs baseline: 1.2573x; 1.2573x over previous
"""Trainium2 Bass kernel for nn_MultiHeadAttention_55894704390646.

Multi-head causal attention, B=2, S=2048, E=1024, H=16 heads, D=64.
Sharding: data-parallel over batch (2 groups) x tensor-parallel over heads
(4 heads per core). Each core computes a partial output-projection result
(row-split Wo); the host sums the 4 partials per batch and adds the bias.

v2 design (all matmul operands bf16, f32 PSUM accumulation):
  - host supplies x^T [E, S] and pre-transposed weight slices in bf16, so
    every matmul contraction dim lands on SBUF partitions with no on-device
    transposes, and input DMA bytes are halved.
  - scores computed transposed ([keys, queries]); softmax uses exp(s/8)
    (no max subtraction: |s/8| is bounded) and the denominator comes from a
    ones-column appended to v (lhsT free size 65).
  - causal narrowing: diagonal-region k-blocks only compute/exp/AV the
    live query range [128*i, 512); a single [128,128] triangle mask zeroes
    the partial block via one bf16 DVE multiply.
  - v / next-chunk q,k / prev-chunk output-projection matmuls are emitted
    as *fillers* between attention groups so the PE queue never drains
    while exp latency is being covered, and short-M matmuls hide their
    LDWEIGHTS under neighboring 512-row streams.
  - output projection is delayed one pair-phase so the SBUF shift DMA that
    places head-1 context into partitions 64:128 is long landed.
  - engine balance: exp + q/k evac on ACT, masks/recip/normalize/v/out
    evac on DVE, denominator broadcast + shift DMAs on Pool, loads/stores
    on the sync queue in priority order.
"""

import sys

if "/opt/trn_rl_repo" not in sys.path:
    sys.path.insert(0, "/opt/trn_rl_repo")

import numpy as np
import ml_dtypes

import concourse.bass as bass
from concourse import bacc
import concourse.mybir as mybir
import concourse.tile as tile
from concourse.bass_utils import run_bass_kernel_spmd

B, S, E, H, D = 2, 2048, 1024, 16, 64
N_CORES = 8
DP = 2                 # batch groups
TP = 4                 # cores per batch group
HL = H // TP           # local heads per core = 4
DL = HL * D            # local head dims = 256
P = 128
NTB = S // P           # token blocks = 16
QC = 512               # query chunk
NQC = S // QC          # query chunks = 4
NKB = QC // P          # k-blocks per q chunk = 4
NPAIR = HL // 2        # head pairs = 2
NEO = E // QC          # output feature chunks of 512 = 2
NKO = E // P           # contraction blocks over E = 8

f32 = mybir.dt.float32
bf16 = mybir.dt.bfloat16
EXP = mybir.ActivationFunctionType.Exp

_NC_CACHE = None


def _build_nc():
    nc = bacc.Bacc("TRN2", target_bir_lowering=False, debug=False)

    xT = nc.dram_tensor("xT", (E, S), bf16, kind="ExternalInput")
    wqT = nc.dram_tensor("wqT", (E, DL), bf16, kind="ExternalInput")
    wkT = nc.dram_tensor("wkT", (E, DL), bf16, kind="ExternalInput")
    wvT = nc.dram_tensor("wvT", (E, DL), bf16, kind="ExternalInput")
    woT = nc.dram_tensor("woT", (DL, E), bf16, kind="ExternalInput")
    out = nc.dram_tensor("out", (S, E), bf16, kind="ExternalOutput")

    with tile.TileContext(nc) as tc:
        with (
            nc.allow_low_precision(reason="bf16 operands / f32 accumulation"),
            tc.tile_pool(name="big", bufs=1) as big,
            tc.tile_pool(name="work", bufs=4) as work,
            tc.tile_pool(name="w2", bufs=4) as w2,
            tc.tile_pool(name="ps_s", bufs=2, space="PSUM") as ps_s,
            tc.tile_pool(name="ps_ctx", bufs=2, space="PSUM") as ps_ctx,
            tc.tile_pool(name="ps_mm", bufs=2, space="PSUM") as ps_mm,
        ):
            # ---- input loads (sync queue), priority order ----------------
            # per-ko weight tiles + per-(ko, chunk) x tiles so the first
            # projection chain starts as soon as its first pieces land.
            wq_t = [big.tile([P, DL], bf16, tag=f"wq{ko}", name=f"wq{ko}") for ko in range(NKO)]
            wk_t = [big.tile([P, DL], bf16, tag=f"wk{ko}", name=f"wk{ko}") for ko in range(NKO)]
            wv_t = [big.tile([P, DL], bf16, tag=f"wv{ko}", name=f"wv{ko}") for ko in range(NKO)]
            xq = [
                [big.tile([P, QC], bf16, tag=f"x{ko}_{c}", name=f"x{ko}_{c}") for c in range(NQC)]
                for ko in range(NKO)
            ]
            wqT_r = wqT[:].rearrange("(ko p) d -> ko p d", p=P)
            wkT_r = wkT[:].rearrange("(ko p) d -> ko p d", p=P)
            wvT_r = wvT[:].rearrange("(ko p) d -> ko p d", p=P)
            xT_r = xT[:].rearrange("(ko p) (c s) -> ko p c s", p=P, c=NQC)
            for ko in range(NKO):
                nc.sync.dma_start(wq_t[ko][:], wqT_r[ko, :, :])
                nc.sync.dma_start(xq[ko][0][:], xT_r[ko, :, 0, :])
            for ko in range(NKO):
                nc.sync.dma_start(wk_t[ko][:], wkT_r[ko, :, :])
            for ko in range(NKO):
                nc.sync.dma_start(wv_t[ko][:], wvT_r[ko, :, :])
            for ko in range(NKO):
                nc.sync.dma_start(xq[ko][1][:], xT_r[ko, :, 1, :])
            woT_sb = big.tile([P, NPAIR, E], bf16, tag="woT", name="woT")
            nc.sync.dma_start(woT_sb[:], woT[:].rearrange("(pr p) e -> p pr e", p=P))
            for c in (2, 3):
                for ko in range(NKO):
                    nc.sync.dma_start(xq[ko][c][:], xT_r[ko, :, c, :])

            # ---- constants (Pool) ---------------------------------------
            # triangle mask: mask[k, q] = 1 if k <= q else 0
            mask = big.tile([P, P], bf16, tag="mask", name="mask")
            nc.gpsimd.memset(mask[:], 1.0)
            nc.gpsimd.affine_select(
                out=mask[:],
                in_=mask[:],
                compare_op=mybir.AluOpType.is_ge,
                fill=0.0,
                base=0,
                pattern=[[1, P]],
                channel_multiplier=-1,
            )

            # ---- persistent activation tiles ----------------------------
            qT_c = [[None] * NQC for _ in range(NPAIR)]
            kT_c = [[None] * NQC for _ in range(NPAIR)]
            for pr in range(NPAIR):
                for ch in range(NQC):
                    qT_c[pr][ch] = big.tile([P, QC], bf16, tag=f"qT{pr}{ch}", name=f"qT{pr}{ch}")
                    kT_c[pr][ch] = big.tile([P, QC], bf16, tag=f"kT{pr}{ch}", name=f"kT{pr}{ch}")
            v_tb = []
            for tb in range(NTB):
                vt = big.tile([P, HL, D + 1], bf16, tag=f"v{tb}", name=f"v{tb}")
                nc.gpsimd.memset(vt[:, :, D], 1.0)
                v_tb.append(vt)
            ctx_J = [
                big.tile([P, NPAIR, QC], bf16, tag=f"ctxT{J}", name=f"ctxT{J}") for J in range(NQC)
            ]

            # ---- filler generators (one PE matmul per yield) ------------
            def gen_qk(ch):
                for pr in range(NPAIR):
                    for wt, dst in ((wq_t, qT_c), (wk_t, kT_c)):
                        pp = ps_mm.tile([P, QC], f32, tag="mm", name=f"pqk{pr}{ch}")
                        for ko in range(NKO):
                            nc.tensor.matmul(
                                pp[:],
                                wt[ko][:, pr * P : (pr + 1) * P],
                                xq[ko][ch][:],
                                start=(ko == 0),
                                stop=(ko == NKO - 1),
                            )
                            yield
                        nc.scalar.copy(dst[pr][ch][:], pp[:])

            def gen_v(tb0, tb1):
                for tb in range(tb0, tb1):
                    pv_full = ps_mm.tile([P, QC], f32, tag="mm", name=f"pv{tb}")
                    pv = pv_full[:, 0:DL]
                    tsl = slice((tb % NKB) * P, (tb % NKB + 1) * P)
                    for ko in range(NKO):
                        nc.tensor.matmul(
                            pv[:],
                            xq[ko][tb // NKB][:, tsl],
                            wv_t[ko][:],
                            start=(ko == 0),
                            stop=(ko == NKO - 1),
                        )
                        yield
                    nc.vector.tensor_copy(
                        v_tb[tb][:, :, 0:D],
                        pv[:].rearrange("p (h d) -> p h d", h=HL),
                    )

            def gen_out(J):
                for tb in range(NKB * J, NKB * (J + 1)):
                    tsl = slice((tb % NKB) * P, (tb % NKB + 1) * P)
                    for ec in range(NEO):
                        o_ps = ps_mm.tile([P, QC], f32, tag="mm", name=f"o{tb}{ec}")
                        for pr in range(NPAIR):
                            nc.tensor.matmul(
                                o_ps[:],
                                ctx_J[J][:, pr, tsl],
                                woT_sb[:, pr, ec * QC : (ec + 1) * QC],
                                start=(pr == 0),
                                stop=(pr == NPAIR - 1),
                            )
                            yield
                        o_sb = w2.tile([P, QC], bf16, tag="osb", name=f"ob{tb}{ec}")
                        nc.vector.tensor_copy(o_sb[:], o_ps[:])
                        nc.sync.dma_start(
                            out[tb * P : (tb + 1) * P, ec * QC : (ec + 1) * QC],
                            o_sb[:],
                        )

            # filler queue: list of (kind, chunk, generator)
            fillers = []

            def drain(n):
                k = 0
                while fillers and k < n:
                    try:
                        next(fillers[0][2])
                        k += 1
                    except StopIteration:
                        fillers.pop(0)

            def flush(kind, upto):
                i = 0
                while i < len(fillers):
                    if fillers[i][0] == kind and fillers[i][1] <= upto:
                        for _ in fillers[i][2]:
                            pass
                        fillers.pop(i)
                    else:
                        i += 1

            def flush_all():
                while fillers:
                    for _ in fillers[0][2]:
                        pass
                    fillers.pop(0)

            # ---- attention ---------------------------------------------
            def scores_group(pr, J, I):
                """Scores + exp (+ triangle mask) for k-block I of (pr, J).

                Returns (pT, qoff): exp'd probabilities, transposed
                [keys, head, queries], valid for columns [qoff:QC)."""
                kch, ib = divmod(I, NKB)
                ik = slice(ib * P, (ib + 1) * P)
                di = I - NKB * J
                qoff = di * P if di >= 0 else 0
                s = ps_s.tile([P, 2, QC], f32, tag="s", name=f"s{pr}{J}{I}")
                nc.tensor.matmul(
                    s[:, 0, qoff:],
                    kT_c[pr][kch][0:64, ik],
                    qT_c[pr][J][0:64, qoff:],
                    start=True,
                    stop=True,
                )
                nc.tensor.matmul(
                    s[:, 1, qoff:],
                    kT_c[pr][kch][64:128, ik],
                    qT_c[pr][J][64:128, qoff:],
                    start=True,
                    stop=True,
                )
                pT = work.tile([P, 2, QC], bf16, tag="pT", name=f"pT{pr}{J}{I}")
                nc.scalar.activation(pT[:, :, qoff:], s[:, :, qoff:], EXP, scale=0.125)
                if di >= 0:
                    nc.vector.tensor_tensor(
                        pT[:, :, qoff : qoff + P],
                        pT[:, :, qoff : qoff + P],
                        mask[:, None, :].to_broadcast((P, 2, P)),
                        mybir.AluOpType.mult,
                    )
                return pT, qoff

            def normalize(ctx_ps, pr, r, J):
                """ctx_J[h-half] = ctx[0:64] / ctx[64] (bf16).

                r=1 goes through an SBUF shift DMA into partitions 64:128."""
                # reciprocal_approx_fast reads garbage from PSUM (measured on
                # HW) — stage the denominator row through SBUF first.
                dn = w2.tile([1, QC], f32, tag="dn", name=f"dn{pr}{r}{J}")
                nc.vector.tensor_copy(dn[:], ctx_ps[D : D + 1, :])
                rc = w2.tile([1, QC], f32, tag="rc", name=f"rc{pr}{r}{J}")
                nc.vector.reciprocal_approx_fast(rc[:], dn[:])
                rb = w2.tile([64, QC], f32, tag="rb", name=f"rb{pr}{r}{J}")
                nc.gpsimd.partition_broadcast(rb[:], rc[:])
                if r == 0:
                    nc.vector.tensor_tensor(
                        ctx_J[J][0:64, pr, :],
                        ctx_ps[0:D, :],
                        rb[:],
                        mybir.AluOpType.mult,
                    )
                else:
                    tmp = w2.tile([64, QC], bf16, tag="tmp", name=f"ct{pr}{J}")
                    nc.vector.tensor_tensor(
                        tmp[:], ctx_ps[0:D, :], rb[:], mybir.AluOpType.mult
                    )
                    nc.gpsimd.dma_start(ctx_J[J][64:128, pr, :], tmp[:])

            def emit_attn_pair(pr, J):
                """Attention for head pair (2pr, 2pr+1) on query chunk J.

                AVs are skewed one k-block behind the scores so the in-order
                PE queue never waits on exp/mask; fillers emitted between
                groups keep the PE dense and deepen the skew."""
                h0, h1 = 2 * pr, 2 * pr + 1
                nI = NKB * (J + 1)
                ctx0 = ps_ctx.tile([D + 1, QC], f32, tag="ctx", name=f"c0_{pr}{J}")
                ctx1 = ps_ctx.tile([D + 1, QC], f32, tag="ctx", name=f"c1_{pr}{J}")

                def emit_av(I, pT, qoff):
                    nc.tensor.matmul(
                        ctx0[:, qoff:], v_tb[I][:, h0, :], pT[:, 0, qoff:],
                        start=(I == 0), stop=(I == nI - 1),
                    )
                    nc.tensor.matmul(
                        ctx1[:, qoff:], v_tb[I][:, h1, :], pT[:, 1, qoff:],
                        start=(I == 0), stop=(I == nI - 1),
                    )

                prev = pending.pop() if pending else scores_group(pr, J, 0)
                for I in range(1, nI):
                    cur = scores_group(pr, J, I)
                    drain(4)
                    emit_av(I - 1, *prev)
                    drain(3)
                    prev = cur
                # prefetch the NEXT pair's first scores group before the last
                # AV + normalize so the PE queue never drains at pair starts
                nxt = chain.pop(0) if chain else None
                if nxt is not None:
                    if nxt[1] != J:
                        flush("qk", nxt[1])
                    pending.append(scores_group(nxt[0], nxt[1], 0))
                emit_av(nI - 1, *prev)
                normalize(ctx1, pr, 1, J)
                normalize(ctx0, pr, 0, J)

            # ---- schedule ----------------------------------------------
            chain = [(0, 0), (1, 0), (0, 1), (1, 1), (0, 2), (1, 2), (0, 3), (1, 3)]
            pending = []
            chain.pop(0)

            for _ in gen_qk(0):
                pass
            for _ in gen_v(0, NKB):
                pass
            fillers.append(("qk", 1, gen_qk(1)))
            fillers.append(("v", 1, gen_v(NKB, 2 * NKB)))
            for J in range(NQC):
                flush("v", J)
                emit_attn_pair(0, J)
                emit_attn_pair(1, J)
                if J + 2 <= NQC - 1:
                    fillers.append(("qk", J + 2, gen_qk(J + 2)))
                    fillers.append(
                        ("v", J + 2, gen_v(NKB * (J + 2), NKB * (J + 3)))
                    )
                if J < NQC - 1:
                    fillers.append(("out", J, gen_out(J)))
            flush_all()
            for _ in gen_out(NQC - 1):
                pass

    nc.compile()
    return nc


def get_nc():
    global _NC_CACHE
    if _NC_CACHE is None:
        _NC_CACHE = _build_nc()
    return _NC_CACHE


def make_in_maps(x, Wq, Wk, Wv, Wo):
    bf = ml_dtypes.bfloat16
    x = np.asarray(x, dtype=np.float32)
    Wq = np.asarray(Wq, dtype=np.float32)
    Wk = np.asarray(Wk, dtype=np.float32)
    Wv = np.asarray(Wv, dtype=np.float32)
    Wo = np.asarray(Wo, dtype=np.float32)
    in_maps = []
    for c in range(N_CORES):
        b, g = divmod(c, TP)
        sl = slice(DL * g, DL * (g + 1))
        in_maps.append(
            {
                "xT": np.ascontiguousarray(x[b].T).astype(bf),
                "wqT": np.ascontiguousarray(Wq[sl].T).astype(bf),
                "wkT": np.ascontiguousarray(Wk[sl].T).astype(bf),
                "wvT": np.ascontiguousarray(Wv[sl].T).astype(bf),
                "woT": np.ascontiguousarray(Wo[:, sl].T).astype(bf),
            }
        )
    return in_maps


def _combine(results, bo):
    bo = np.asarray(bo, dtype=np.float32)
    y = np.zeros((B, S, E), dtype=np.float32)
    for c in range(N_CORES):
        y[c // TP] += results[c]["out"].astype(np.float32)
    y += bo
    return y


def kernel(x, Wq, Wk, Wv, Wo, bo):
    nc = get_nc()
    in_maps = make_in_maps(x, Wq, Wk, Wv, Wo)
    res = run_bass_kernel_spmd(nc, in_maps, list(range(N_CORES)))
    return _combine(res.results, bo)


def kernel_traced(x, Wq, Wk, Wv, Wo, bo, trace_cores=None):
    """Like kernel() but with NTFF tracing; returns (output, BassKernelResults)."""
    nc = get_nc()
    in_maps = make_in_maps(x, Wq, Wk, Wv, Wo)
    res = run_bass_kernel_spmd(
        nc, in_maps, list(range(N_CORES)), trace=True, trace_cores=trace_cores
    )
    return _combine(res.results, bo), res
